# revision 1
# baseline (speedup 1.0000x reference)
"""Dilated attention (LongNet-style) Trainium2 Bass kernel.

Problem: q/k/v [b=2, seq=8192, h=12, d=64], 3 dilation groups of 4 heads:
  group 0: segment 2048, rate 1, off 0, heads 0-3   -> 4 segments/batch
  group 1: segment 4096, rate 2, off 1, heads 4-7   -> 2 segments/batch
  group 2: segment 8192, rate 4, off 2, heads 8-11  -> 1 segment/batch
Every (batch, head, segment) is an independent causal attention of shape
[m=2048, k=2048, d=64]; there are 56 such problems (32+16+8), all equal cost.

Sharding: 8 cores = 2 batches x 4 "head triples". Core c owns batch c//4 and
heads {j, 4+j, 8+j} (j = c%4) -> 4+2+1 = 7 problems per core, and every head
lives entirely on one core, so the final seq-sum renormalization is local
(no collectives).

On-core layout ("transposed"): S^T[k, m] = K Q^T computed per (k-chunk=128,
m-tile=512); exp via ACT; PV accumulates O^T[d, m] with lhsT = V_aug
([128, 65], last column ones => row 64 of O^T is the softmax denominator l[m]).
Causality: k-chunks fully above the diagonal are skipped, band chunks are
column-trimmed and their leading 128x128 triangle is zeroed on the P tile
after exp. d sits on partitions in O^T, so the per-head seq-sum renorm is a
free-axis reduce + per-partition scalar multiply.

Numerics: everything through the matmuls is fp32 (4 cycles/row on PE). The
final seq-sum renorm divides by a heavily-cancelled sum, amplifying relative
error ~1000x for some heads; fp32r/bf16 products fail by orders of magnitude
(measured), so fp32 it is.

PE optimization: the QK^T matmuls contract over d=64 (half the PE array), so
q/k are duplicated onto both partition halves and consecutive k-chunks are
issued as adjacent matmuls on row groups (0,0)/(64,0) -> they run
concurrently (measured exact on HW), ~2x the S throughput. Per m-tile the
schedule is phase-split (all S pairs, then all PVs) so S pairs stay adjacent
in PE order; P tiles wait in SBUF.

Wait-fan-in: Bacc's generate_event_semaphores splits multi-sem waits, but the
structure still keeps wait fan-in low (single qk DMA, upfront V DMA, single
releasing engine per PSUM pool).
"""

import numpy as np

B, SEQ, H, D = 2, 8192, 12, 64
NP = 7            # problems per core
M = 2048          # dilated positions per problem
MT = 512          # m-tile width
KC = 128          # k-chunk (partition) width
NMT = M // MT     # 4 m-tiles
NKC = M // KC     # 16 k-chunks
SCALE = 0.125     # 1/sqrt(64)

_CACHE = {}


def _core_problems(core):
    """The 7 (head, positions) problems for a core; batch = core//4."""
    j = core % 4
    probs = []
    for p in range(4):
        probs.append((j, p * 2048 + np.arange(2048)))
    for p in range(2):
        probs.append((4 + j, p * 4096 + 1 + 2 * np.arange(2048)))
    probs.append((8 + j, 2 + 4 * np.arange(2048)))
    return probs


# head -> list of problem indices on its core
HEAD_GROUPS = ((0, 1, 2, 3), (4, 5), (6,))


def _import_concourse():
    try:
        import concourse  # noqa: F401
    except ImportError:
        import sys

        for p in ("/opt/trn_rl_repo", "/root/.axon_site/_ro/trn_rl_repo"):
            if p not in sys.path:
                sys.path.append(p)


def _build_program(causal, reps=1):
    """Build the SPMD program. reps>1 wraps the compute in a hardware loop
    (timing-only variant; the deliverable path uses reps=1)."""
    _import_concourse()
    import contextlib

    import concourse.bass as bass  # noqa: F401
    import concourse.tile as tile
    from concourse import bacc, mybir

    F32 = mybir.dt.float32

    nc = bacc.Bacc()

    # q and k share one tensor: [p, :, 0:2048]=Q^T, [p, :, 2048:4096]=K^T.
    # DMA'd twice (partitions 0:64 and 64:128) so even k-chunks run on PE row
    # group 0 and odd chunks on row group 64.
    qkT_d = nc.dram_tensor("qkT", [NP, D, 2 * M], F32, kind="ExternalInput")
    vA_d = nc.dram_tensor("vA", [KC, NP, NKC, D + 1], F32, kind="ExternalInput")
    out_d = nc.dram_tensor("out", [NP, D, M], F32, kind="ExternalOutput")

    # additive causal mask for the leading 128x128 triangle of band chunks:
    # 0 where col>=row (valid), -1e9 otherwise (exp underflows to exactly 0).
    # Applied to the scores in PSUM *before* exp so the P tiles have a pure
    # ACT->PE chain: exp then needs only a single semaphore wait (no EVSEM),
    # keeping ACT throughput at the packed-PE feed rate.
    mneg = np.where(
        np.arange(KC)[None, :] >= np.arange(KC)[:, None], 0.0, -1e9
    ).astype(np.float32)
    mask_d = nc.inline_tensor(mneg, name="cmask")

    with tile.TileContext(nc) as tc:
        with (
            tc.tile_pool(name="qk", bufs=4) as qk_pool,
            tc.tile_pool(name="pt", bufs=17) as p_pool,
            tc.tile_pool(name="small", bufs=1) as small_pool,
            tc.tile_pool(name="stage", bufs=1) as stage_pool,
            tc.tile_pool(name="rl", bufs=3) as rl_pool,
            tc.tile_pool(name="bc", bufs=3) as bc_pool,
            tc.tile_pool(name="spsum", bufs=5, space="PSUM") as s_psum,
            tc.tile_pool(name="opsum", bufs=2, space="PSUM") as o_psum,
            tc.tile_pool(name="bpsum", bufs=1, space="PSUM") as b_psum,
        ):
            mask_sb = small_pool.tile([KC, KC], F32)
            nc.sync.dma_start(out=mask_sb, in_=mask_d[:])
            ones_sb = small_pool.tile([1, D], F32)
            nc.vector.memset(ones_sb, 1.0)

            # all value tensors upfront; split per problem AND per
            # partition-slab so transfers spread across HWDGE queues
            # (per-queue bandwidth is ~10-15 GB/s)
            va = small_pool.tile([KC, NP, NKC, D + 1], F32)
            for p in range(NP):
                for s in range(4):
                    sl = slice(s * 32, (s + 1) * 32)
                    nc.sync.dma_start(
                        out=va[sl, p, :, :], in_=vA_d[sl, p, :, :]
                    )

            # O_norm^T staging: [d=64, problem, m=2048]
            stage = stage_pool.tile([D, NP, M], F32)
            # per (problem, m-tile) partial seq-sums
            dsums = small_pool.tile([D, NP * NMT], F32)

            # head renorm emitted as soon as its problems complete, so the
            # tail DVE/DMA work overlaps later problems' PE work
            done_after = {3: HEAD_GROUPS[0], 5: HEAD_GROUPS[1], 6: HEAD_GROUPS[2]}

            rep_loop = (
                tc.For_i(0, reps, 1) if reps > 1 else contextlib.nullcontext()
            )
            with rep_loop:
              for p in range(NP):
                # 8 slab DMAs (4 per duplicated half) to spread descriptor
                # latency across HWDGE queues
                qkt = qk_pool.tile([2 * D, 2 * M], F32, tag="qkt")
                SL = D // 4
                for h in range(2):
                    for s in range(4):
                        nc.sync.dma_start(
                            out=qkt[h * D + s * SL:h * D + (s + 1) * SL, :],
                            in_=qkT_d[p, s * SL:(s + 1) * SL, :],
                        )

                for j in range(NMT):
                    m0 = j * MT
                    if causal:
                        # band chunks (4j..4j+3, trimmed) first so their
                        # serial S->mask->exp chains hide behind the full
                        # chunks' matmuls; band 4j is full-width, so the
                        # PV accumulation start still covers the whole bank
                        chunks = [(4 * j + i, KC * i) for i in range(4)]
                        chunks += [(kc, 0) for kc in range(4 * j)]
                    else:
                        chunks = [(kc, 0) for kc in range(NKC)]

                    # phase 1: all S matmuls as row-group pairs + exp + mask
                    pts = []
                    for idx, (kc, coff) in enumerate(chunks):
                        w = MT - coff
                        half = idx % 2  # row group: even chunk -> 0, odd -> 64
                        r0 = half * D
                        st = s_psum.tile([KC, MT], F32, tag="st")
                        nc.tensor.matmul(
                            st[:, :w],
                            qkt[r0:r0 + D, M + kc * KC:M + (kc + 1) * KC],
                            qkt[r0:r0 + D, m0 + coff:m0 + MT],
                            start=True,
                            stop=True,
                        )
                        if causal and kc >= 4 * j:
                            # mask the triangle of this band chunk pre-exp
                            nc.vector.tensor_add(
                                st[:, :KC], st[:, :KC], mask_sb
                            )
                        pt = p_pool.tile([KC, MT], F32, tag="pt")
                        nc.scalar.activation(
                            pt[:, :w],
                            st[:, :w],
                            mybir.ActivationFunctionType.Exp,
                            scale=SCALE,
                        )
                        pts.append((pt, kc, coff, w))

                    # phase 2: PV accumulation over all chunks
                    ot = o_psum.tile([D + 1, MT], F32)
                    for idx, (pt, kc, coff, w) in enumerate(pts):
                        nc.tensor.matmul(
                            ot[:, coff:],
                            va[:, p, kc, :],
                            pt[:, :w],
                            start=(idx == 0),
                            stop=(idx == len(pts) - 1),
                        )

                    # softmax normalization: divide by l[m] (row 64 of ot).
                    # 1/l is broadcast across partitions with a K=1 fp32 PE
                    # outer product (exact; all on-chip, ~1us latency -- a
                    # DMA-based broadcast costs ~100us of descriptor latency
                    # on this critical chain and stalls the PSUM pipeline).
                    rl = rl_pool.tile([1, MT], F32, tag="rl")
                    nc.vector.reciprocal(rl, ot[D:D + 1, :])
                    bcp = b_psum.tile([D, MT], F32)
                    nc.tensor.matmul(bcp, ones_sb, rl, start=True, stop=True)
                    bc = bc_pool.tile([D, MT], F32, tag="bc")
                    nc.vector.tensor_copy(bc, bcp)
                    dst = stage[:, p, m0:m0 + MT]
                    nc.vector.tensor_mul(dst, ot[:D, :], bc)
                    nc.vector.reduce_sum(
                        out=dsums[:, p * NMT + j:p * NMT + j + 1],
                        in_=dst,
                        axis=mybir.AxisListType.X,
                    )

                # per-head seq-sum renorm + output, as soon as the head is done
                if p in done_after:
                    probs = done_after[p]
                    denom = rl_pool.tile([D, 1], F32, tag="dn")
                    lo, hi = probs[0] * NMT, (probs[-1] + 1) * NMT
                    nc.vector.reduce_sum(
                        out=denom, in_=dsums[:, lo:hi], axis=mybir.AxisListType.X
                    )
                    rden = rl_pool.tile([D, 1], F32, tag="rd")
                    nc.vector.reciprocal(rden, denom)
                    for pp in probs:
                        nc.vector.tensor_scalar_mul(
                            stage[:, pp, :], stage[:, pp, :], rden
                        )
                        # 4 slab DMAs -> 4 parallel queues, issued from the
                        # (otherwise idle) gpsimd queue so output transfers
                        # never serialize behind the sync queue's input
                        # prefetch slot-waits (SP is strict FIFO)
                        for s in range(4):
                            sl = slice(s * 16, (s + 1) * 16)
                            nc.gpsimd.dma_start(
                                out=out_d[pp, sl, :], in_=stage[sl, pp, :]
                            )

    nc.finalize()
    return nc


def _shard_inputs(query, key, value):
    """Build the 8 per-core input maps from full inputs."""
    in_maps = []
    for core in range(8):
        b = core // 4
        qkT = np.empty((NP, D, 2 * M), np.float32)
        vA = np.empty((KC, NP, NKC, D + 1), np.float32)
        vA[..., D] = 1.0
        for p, (h, pos) in enumerate(_core_problems(core)):
            s, e, st = pos[0], pos[-1] + 1, (pos[1] - pos[0])
            qkT[p, :, :M] = query[b, s:e:st, h, :].T
            qkT[p, :, M:] = key[b, s:e:st, h, :].T
            # vA[i, p, c, :64] = V[c*128 + i]
            vA[:, p, :, :D] = value[b, s:e:st, h, :].reshape(NKC, KC, D).transpose(1, 0, 2)
        in_maps.append({"qkT": qkT, "vA": vA})
    return in_maps


def _unshard(results):
    out = np.zeros((B, SEQ, H, D), np.float32)
    for core in range(8):
        b = core // 4
        o = results[core]["out"]  # [NP, 64, 2048]
        for p, (h, pos) in enumerate(_core_problems(core)):
            s, e, st = pos[0], pos[-1] + 1, (pos[1] - pos[0])
            out[b, s:e:st, h, :] = o[p].T
    return out


def kernel(query, key, value, causal):
    _import_concourse()
    from concourse.bass_utils import run_bass_kernel_spmd

    query = np.asarray(query, np.float32)
    key = np.asarray(key, np.float32)
    value = np.asarray(value, np.float32)
    causal = bool(int(np.asarray(causal)))

    if causal not in _CACHE:
        _CACHE[causal] = _build_program(causal)
    nc = _CACHE[causal]

    in_maps = _shard_inputs(query, key, value)
    res = run_bass_kernel_spmd(nc, in_maps, core_ids=list(range(8)))
    return _unshard(res.results)

